# revision 39
# baseline (speedup 1.0000x reference)
"""BitLinear (ternary-quantized linear) Trainium2 kernel.

out = x @ (gamma * ternary(weight)).T + bias, computed tensor-parallel over
8 NeuronCores: weight/bias sharded along out_features, x replicated.

Per-core device program (v5 — no PE transposes, supply-first queue order):
  1. Cast x (fp32) -> bf16 into DRAM scratch via SWDGE casting DMAs,
     throttled so the 32MB fp32 weight stream owns HBM during the prologue.
  2. The weight shard arrives HOST-TRANSPOSED as wT [K, NS] fp32 (layout-only
     host change; quantization happens on device in fp32, preserving the
     exact ternary boundary). [128 k, 512 n] chunks stream nb-major, loads
     alternating across both HWDGE queues, and are quantized elementwise to
     doubled ternary {-2,0,2} bf16 straight into the SBUF-resident
     [K-partition, k-subtile, NS] tile wqT:
       even chunks on ACT:  q' = sign(w - thr) + sign(w + thr), add on DVE
       odd  chunks on DVE:  q' = 2*(w >= thr) - 2*(w <= -thr)
     with thr = 0.5*gamma (factor 2 folded into the gamma/2 output scale).
     All supply-side instructions are emitted before any drain so no queue
     ever blocks on a compute-completion semaphore ahead of supply work.
  3. Prologue compute: the PE processes m-tiles 0..3 one 512-wide n-block
     stage at a time, chasing the quantizer chunk-by-chunk (4 PSUM banks per
     stage, so consecutive stages double-buffer). Steady state (tiles
     4..63) is the per-tile kt-outer / nb-inner loop against the fully
     resident wqT; the last tile runs nb-outer so its drains overlap its
     own matmuls.
  4. Drains: prologue stages evict+bias on DVE with outputs on the Sync
     queue (threaded between steady-state transposes); steady tiles evict
     on ACT (Copy activation, per-partition gamma/2 scale), bias on DVE,
     outputs on the Scalar queue.

gamma = max(mean(|clip(w, -2, 2)|), 1e-4) is a global scalar over the full
weight; it is computed on host with the same jnp ops the module uses so the
quantization boundary matches bit-exactly, and enters the device kernel as a
[128, 4] scalar input tensor (threshold, -threshold, gamma/2).
"""

import numpy as np

import concourse.bass as bass
import concourse.mybir as mybir
import concourse.tile as tile
from concourse import bacc
from concourse.bass_utils import run_bass_kernel_spmd
from concourse.tile import add_dep_helper

P = 128
B, S, D_IN, D_OUT = 4, 2048, 4096, 16384
M = B * S                 # 8192 tokens
K = D_IN                  # 4096 contraction
N_CORES = 8
NS = D_OUT // N_CORES     # 2048 out-features per core
KT = K // P               # 32 k-subtiles
MT = M // P               # 64 m-tiles
NBS = 512                 # psum bank free size (fp32)
NB = NS // NBS            # 4 psum n-blocks

F32 = mybir.dt.float32
BF16 = mybir.dt.bfloat16

_NC_CACHE = None
LAST_RESULTS = None


def _build_nc():
    nc = bacc.Bacc(None, target_bir_lowering=False, debug=False)

    x_in = nc.declare_dram_parameter("x", [M, K], F32, isOutput=False)
    w_in = nc.declare_dram_parameter("w", [K, NS], F32, isOutput=False)
    b_in = nc.declare_dram_parameter("bias", [P, NS], F32, isOutput=False)
    s_in = nc.declare_dram_parameter("scal", [P, 4], F32, isOutput=False)
    y_out = nc.declare_dram_parameter("out", [M, NS], F32, isOutput=True)

    CAST_AHEAD = 6
    PRO_TILES = 4     # m-tiles processed stage-wise during the prologue

    with tile.TileContext(nc) as tc:
        with (
            tc.tile_pool(name="const", bufs=1) as constp,
            tc.tile_pool(name="w_sb", bufs=6) as wsbp,
            tc.tile_pool(name="qab", bufs=6) as qabp,
            tc.tile_pool(name="xT", bufs=5) as xTp,
            tc.tile_pool(name="osb", bufs=6) as osbp,
            tc.tile_pool(name="psum", bufs=8, space="PSUM") as psump,
            tc.tile_pool(name="dram", bufs=1, space="DRAM") as dramp,
        ):
            scal = constp.tile([P, 4], F32)
            nc.sync.dma_start(out=scal[:], in_=s_in[:])
            bias_sb = constp.tile([P, NS], F32)
            nc.sync.dma_start(out=bias_sb[:], in_=b_in[:])
            # full quantized-transposed weight shard, resident in SBUF
            wqT = constp.tile([P, KT, NS], BF16)

            # ---- x fp32 -> bf16 cast, DRAM->DRAM on SWDGE ----
            xhat = []
            cast_insts = []
            for j in range(MT):
                xh = dramp.tile([P, K], BF16, name=f"xhat_{j}")
                if j < PRO_TILES:
                    # sliver the first tiles so each transpose can start as
                    # soon as possible behind the serial SWDGE stream
                    ci = None
                    for s in range(4):
                        r0, r1 = s * 32, (s + 1) * 32
                        ci = nc.gpsimd.dma_start(
                            out=xh[r0:r1, :], in_=x_in[j * P + r0:j * P + r1, :]
                        )
                else:
                    ci = nc.gpsimd.dma_start(
                        out=xh[:], in_=x_in[j * P:(j + 1) * P, :]
                    )
                xhat.append(xh)
                cast_insts.append(ci)

            def emit_xread(j, xT):
                if j < 1:
                    xr = None
                    for s in range(4):
                        r0, r1 = s * 32, (s + 1) * 32
                        xr = nc.sync.dma_start_transpose(
                            xT[:, :, r0:r1], xhat[j][r0:r1, :]
                        )
                else:
                    xr = nc.sync.dma_start_transpose(xT[:], xhat[j][:])
                if j + CAST_AHEAD < MT:
                    add_dep_helper(
                        cast_insts[j + CAST_AHEAD].ins,
                        xr.ins,
                        reason="throttle x-cast to stay a few m-tiles ahead",
                    )
                return xr

            pro_xTs = [
                xTp.tile([P, KT, P], BF16, tag="xT", name=f"xT_{j}")
                for j in range(PRO_TILES)
            ]

            def emit_wload(nb, kt, cidx):
                w_sb = wsbp.tile([P, NBS], F32, tag="w_in")
                dq = nc.sync if (cidx % 2 == 0) else nc.scalar
                wl = dq.dma_start(
                    out=w_sb[:],
                    in_=w_in[kt * P:(kt + 1) * P, nb * NBS:(nb + 1) * NBS],
                )
                return w_sb, wl

            def emit_quant(nb, kt, cidx, w_sb):
                if cidx % 2 == 0:
                    sa = qabp.tile([P, NBS], BF16, tag="q")
                    sb = qabp.tile([P, NBS], BF16, tag="q")
                    nc.scalar.sign(sa[:], w_sb[:], bias=scal[:, 1:2])  # -thr
                    nc.scalar.sign(sb[:], w_sb[:], bias=scal[:, 0:1])  # +thr
                    nc.vector.tensor_tensor(
                        wqT[:, kt, nb * NBS:(nb + 1) * NBS],
                        sa[:], sb[:], mybir.AluOpType.add,
                    )
                else:
                    ga = qabp.tile([P, NBS], BF16, tag="q")
                    gb = qabp.tile([P, NBS], BF16, tag="q")
                    nc.vector.tensor_scalar(
                        ga[:], w_sb[:], scal[:, 0:1], 2.0,
                        mybir.AluOpType.is_ge, mybir.AluOpType.mult,
                    )
                    nc.vector.tensor_scalar(
                        gb[:], w_sb[:], scal[:, 1:2], 2.0,
                        mybir.AluOpType.is_le, mybir.AluOpType.mult,
                    )
                    nc.vector.tensor_tensor(
                        wqT[:, kt, nb * NBS:(nb + 1) * NBS],
                        ga[:], gb[:], mybir.AluOpType.subtract,
                    )

            # ---- supply emission: all 128 weight chunks, nb-major ----
            # x transposes for prologue tiles are threaded between chunk
            # groups at the points they're needed.
            for nb in range(NB):
                for kt in range(KT):
                    cidx = nb * KT + kt
                    w_sb, wl = emit_wload(nb, kt, cidx)
                    emit_quant(nb, kt, cidx, w_sb)
                    if cidx == 7:
                        emit_xread(0, pro_xTs[0])
                    elif cidx == 31:
                        emit_xread(1, pro_xTs[1])
                    elif cidx == 47:
                        emit_xread(2, pro_xTs[2])
                    elif cidx == 63:
                        emit_xread(3, pro_xTs[3])

            # ---- prologue stages: tiles 0..3 against each n-block ----
            pro_drains = []
            for nb in range(NB):
                pss = [
                    psump.tile([P, NBS], F32, tag="ps", name=f"ps_{j}_{nb}")
                    for j in range(PRO_TILES)
                ]
                if nb in (0, NB - 1):
                    # tile-major: stage 0 because later tiles' x is still
                    # landing; the last stage so the xT buffers release
                    # early for the first steady-state transposes
                    order = [(j, kt) for j in range(PRO_TILES) for kt in range(KT)]
                else:
                    order = [(j, kt) for kt in range(KT) for j in range(PRO_TILES)]
                for j, kt in order:
                    nc.tensor.matmul(
                        pss[j][:],
                        pro_xTs[j][:, kt, :],
                        wqT[:, kt, nb * NBS:(nb + 1) * NBS],
                        start=(kt == 0),
                        stop=(kt == KT - 1),
                    )
                # evict + bias on DVE (keeps the ACT sign stream unblocked);
                # the output DMAs are threaded onto the Sync queue later
                stage_osbs = []
                for j in range(PRO_TILES):
                    osb = osbp.tile([P, NBS], F32, tag="osb", name=f"osb_{j}_{nb}")
                    nc.vector.tensor_scalar(
                        osb[:], pss[j][:], scal[:, 2:3], None,
                        mybir.AluOpType.mult,
                    )
                    nc.vector.tensor_tensor(
                        osb[:], osb[:], bias_sb[:, nb * NBS:(nb + 1) * NBS],
                        mybir.AluOpType.add,
                    )
                    stage_osbs.append((j, osb))
                pro_drains.append(stage_osbs)

            def emit_pro_outs(nb):
                for j, osb in pro_drains[nb]:
                    nc.sync.dma_start(
                        out=y_out[j * P:(j + 1) * P, nb * NBS:(nb + 1) * NBS],
                        in_=osb[:],
                    )

            # ---- steady state: per-tile kt-outer / nb-inner ----
            for j in range(PRO_TILES, MT):
                xT = xTp.tile([P, KT, P], BF16, tag="xT", name=f"xT_{j}")
                # thread the prologue output DMAs between steady transposes
                if j - PRO_TILES < NB:
                    emit_pro_outs(j - PRO_TILES)
                emit_xread(j, xT)
                psums = [
                    psump.tile([P, NBS], F32, tag="ps", name=f"ps_{j}_{nb}")
                    for nb in range(NB)
                ]
                last = j == MT - 1
                if last:
                    # nb-outer: drains overlap this tile's own matmuls
                    for nb in range(NB):
                        for kt in range(KT):
                            nc.tensor.matmul(
                                psums[nb][:], xT[:, kt, :],
                                wqT[:, kt, nb * NBS:(nb + 1) * NBS],
                                start=(kt == 0), stop=(kt == KT - 1),
                            )
                        osb = osbp.tile([P, NBS], F32, tag="osb", name=f"osb_{j}_{nb}")
                        nc.scalar.activation(
                            osb[:], psums[nb][:],
                            mybir.ActivationFunctionType.Copy, 0.0, scal[:, 2:3],
                        )
                        nc.vector.tensor_tensor(
                            osb[:], osb[:], bias_sb[:, nb * NBS:(nb + 1) * NBS],
                            mybir.AluOpType.add,
                        )
                        nc.scalar.dma_start(
                            out=y_out[j * P:(j + 1) * P, nb * NBS:(nb + 1) * NBS],
                            in_=osb[:],
                        )
                else:
                    for kt in range(KT):
                        for nb in range(NB):
                            nc.tensor.matmul(
                                psums[nb][:], xT[:, kt, :],
                                wqT[:, kt, nb * NBS:(nb + 1) * NBS],
                                start=(kt == 0), stop=(kt == KT - 1),
                            )
                    for nb in range(NB):
                        osb = osbp.tile([P, NBS], F32, tag="osb", name=f"osb_{j}_{nb}")
                        nc.scalar.activation(
                            osb[:], psums[nb][:],
                            mybir.ActivationFunctionType.Copy, 0.0, scal[:, 2:3],
                        )
                        nc.vector.tensor_tensor(
                            osb[:], osb[:], bias_sb[:, nb * NBS:(nb + 1) * NBS],
                            mybir.AluOpType.add,
                        )
                        nc.scalar.dma_start(
                            out=y_out[j * P:(j + 1) * P, nb * NBS:(nb + 1) * NBS],
                            in_=osb[:],
                        )

    nc.compile()
    return nc


def _compute_gamma(weight: np.ndarray) -> np.float32:
    """Replicate the module's gamma computation bit-exactly (jnp, fp32)."""
    import jax
    import jax.numpy as jnp

    with jax.default_device(jax.devices("cpu")[0]):
        w_f32 = jnp.clip(jnp.asarray(weight, dtype=jnp.float32), -2.0, 2.0)
        gamma = jnp.maximum(jnp.mean(jnp.abs(w_f32)), 1e-4)
        return np.float32(np.asarray(gamma))


def kernel(x: np.ndarray, weight: np.ndarray, bias: np.ndarray) -> np.ndarray:
    global _NC_CACHE, LAST_RESULTS

    x2d = np.ascontiguousarray(np.asarray(x, dtype=np.float32).reshape(M, K))
    weight = np.ascontiguousarray(np.asarray(weight, dtype=np.float32))
    bias = np.asarray(bias, dtype=np.float32)

    gamma = _compute_gamma(weight)
    thr = np.float32(np.float32(0.5) * gamma)
    scal = np.zeros((P, 4), dtype=np.float32)
    scal[:, 0] = thr
    scal[:, 1] = -thr
    scal[:, 2] = np.float32(np.float32(0.5) * gamma)  # psum carries 2x ternary

    if _NC_CACHE is None:
        _NC_CACHE = _build_nc()
    nc = _NC_CACHE

    in_maps = []
    for i in range(N_CORES):
        # host-side layout change only: shard along out_features, then
        # transpose to [K, NS] so device quantization is purely elementwise
        w_shard_T = np.ascontiguousarray(weight[i * NS:(i + 1) * NS].T)
        b_shard = np.ascontiguousarray(
            np.broadcast_to(bias[i * NS:(i + 1) * NS], (P, NS))
        )
        in_maps.append({"x": x2d, "w": w_shard_T, "bias": b_shard, "scal": scal})

    res = run_bass_kernel_spmd(nc, in_maps, list(range(N_CORES)))
    LAST_RESULTS = res

    out = np.concatenate([res.results[i]["out"] for i in range(N_CORES)], axis=1)
    return np.ascontiguousarray(out.reshape(B, S, D_OUT))
